# revision 23
# baseline (speedup 1.0000x reference)
"""Bilateral filter 3x3 (sigma_space = sigma_color = 0.8) on 8 TRN2 NeuronCores.

Sharding: pure data parallelism - one batch image [3, 512, 512] per core.

Math (per core), with the color-normalization cancelled:
  out = c + A/den
    den(x) = ws0 + sum_{k in HP} [G_k(x) + G_k(x-k)]
    A(x)   =       sum_{k in HP} [H_k(x) - H_k(x-k)]
  where HP = {E=(0,1), S=(1,0), SE=(1,1), SW=(1,-1)},
    D_k = p~(x+k) - p(x),  G_k = ws_k * exp(-D_k^2 / (2 s^2)),  H_k = D_k * G_k.

Perf design (~107.9us f32 baseline -> ~80us profiled here):
  - fp16 on-chip: host converts + transposes input to [H,C,W] fp16; output is
    fp16, upcast on the host.  DVE tensor_tensor runs in 2x_1P packed mode,
    HBM traffic halves, PE streams fp16 rhs at 1 col/cycle.
  - G in ONE ACT pass per offset via Derivative_Erf(D*s) =
    (2/sqrt(pi))*exp(-D^2/(2 sigma^2)); kappa and the per-offset spatial
    weights fold into the PE band matrices of BOTH chains (no Square pass,
    no exp bias, no extra muls).  The erf_derivative table is preloaded
    during the DMA ramp by a throwaway activation.
  - rows are loaded at both column parities (A tiles image@col2, B tiles
    image@col3, derived on DVE with alignment-free 2x_2P copies) so all four
    subs are 4-byte aligned and hit 2x.  GPSIMD handles only tiny pad ops:
    bulk GPSIMD tensor ops run ~3.5ns/elem and their SBUF traffic slows
    concurrent DVE ops ~4x.
  - shifted terms accumulate on the TensorEngine with shift-band matmuls
    into PSUM (20 passes per channel-tile); row seams across 128-row tiles
    use selector bands against the previous tile's G/H; image boundaries use
    reflect-mirror identities applied in the D domain (D odd, G even).
  - evacuation: A copied PSUM->SBUF on ACT, then one fused custom DVE op
    t8 = reciprocal1(den + ws0) * A (bitwise-NOT exponent-flip seed + one
    Newton step, 7 ALU stages, ~0.4% max err), then one packed fp16 add
    y = t8 + center.  No separate reciprocal pass, no ws0*ones matmul.
  - only 2 HBM loads per tile + 1 store (split per-channel on the last tile
    so the final, smallest store launches earliest), all HWDGE DMAs kept to
    ~16 total: more DMAs overflow the 8 round-robin HWDGE completion-sem
    lanes and serialize the whole pipeline (observed 13-33us trigger
    stalls; 20+ DMAs cost tens of us).
  - dependency-ordered engine FIFOs: the S offset (no B-copy, no pad mirror)
    leads every stage so ACT starts early; the a_sb PSUM evac sits AFTER
    DerivErf in the scalar FIFO; the custom-op evac sits AFTER the H muls in
    the vector FIFO; den chains run before A chains on PE so single-buffered
    A banks are released by the evac just in time.
  - 22 dummy matmuls during the ramp keep the PE HAM clock-gate warm
    (2.4 GHz); all 240 production matmuls then issue at ~220 ns.

Layout: partition = image rows (4 tiles x 128 rows), free = (channel, width).
WP=522 cols; A tiles: image at cols 2..513; B tiles: image at cols 3..514.
"""
import math
import numpy as np
from contextlib import ExitStack

import concourse.bacc as bacc
import concourse.tile as tile
from concourse import mybir
from concourse.bass_utils import run_bass_kernel_spmd

F32 = mybir.dt.float32
F16 = mybir.dt.float16
MM_DT = F16
MM_NP = np.float16
AF = mybir.ActivationFunctionType
ALU = mybir.AluOpType

C, H, W = 3, 512, 512
P = 128                      # partitions per row-tile
NT = H // P                  # 4 row-tiles
WP = 522                     # col-padded width
IM0 = 2                      # first image column, A-parity tiles
IMB = 3                      # first image column, B-parity tiles
J0 = IM0 - 1
J1 = IM0
J2 = IM0 + 1

SIG = 0.8
TWO_SIG2 = 2.0 * SIG * SIG   # 1.28
SCALE_SQ = 1.0 / math.sqrt(TWO_SIG2)
KAPPA = 2.0 / math.sqrt(math.pi)     # DerivErf(u) = KAPPA * exp(-u^2)
_w1 = math.exp(-1.0 / TWO_SIG2)
_norm = (1.0 + 2.0 * _w1) ** 2
WS0 = 1.0 / _norm            # center weight
WS_E = _w1 / _norm           # edge
WS_K = _w1 * _w1 / _norm     # corner
WE = WS_E / KAPPA            # band / STT scale for edge offsets
WK = WS_K / KAPPA            # band / STT scale for corner offsets

BAND_NAMES = ["b_iE", "b_isE", "b_iK", "b_sK", "b_selE", "b_selK",
              "b_sel0E", "b_sel0K",
              "b_aiE", "b_aniE", "b_ainsE", "b_aiK", "b_ansK",
              "b_anselE", "b_anselK", "b_asel0E", "b_asel0K", "b_i"]


def _bands_np():
    I = np.eye(P, dtype=np.float32)
    S = np.zeros((P, P), np.float32)
    for m in range(1, P):
        S[m - 1, m] = 1.0          # lhsT[p, m]: out row m <- in row m-1
    sel = np.zeros((P, P), np.float32)
    sel[P - 1, 0] = 1.0            # out row 0 <- in row 127 (prev tile)
    sel0 = np.zeros((P, P), np.float32)
    sel0[0, 0] = 1.0               # out row 0 <- in row 0 (top mirror)
    d = {"b_iE": WE * I, "b_isE": WE * (I + S), "b_iK": WK * I,
         "b_sK": WK * S, "b_selE": WE * sel, "b_selK": WK * sel,
         "b_sel0E": WE * sel0, "b_sel0K": WK * sel0,
         "b_aiE": WE * I, "b_aniE": -WE * I, "b_ainsE": WE * (I - S),
         "b_aiK": WK * I, "b_ansK": -WK * S,
         "b_anselE": -WE * sel, "b_anselK": -WK * sel,
         "b_asel0E": WE * sel0, "b_asel0K": WK * sel0, "b_i": I}
    return np.stack([d[k] for k in BAND_NAMES], axis=1)  # [P, 14, P]


# ------------- custom DVE op: t8 = recip1(den + ws0) * A -------------------
_RECIP_OP = None


def _get_recip_op():
    """out = y1 * in1 with y1 = one-Newton-step reciprocal of (in0 + s0)."""
    global _RECIP_OP
    if _RECIP_OP is not None:
        return _RECIP_OP
    from concourse import dve_ops as dvo
    from concourse.dve_spec import Spec, Src0, Src1, C0, C1, C2, AluOp, Bin, lower
    from concourse.dve_uop import DveOpSpec

    name = "RECIP1MUL_WS0_ANT"
    xs = Src0 + C0
    nx = Bin(AluOp.BITWISE_NOT, xs, xs)
    y0 = nx * C1
    y1 = y0 * (C2 - xs * y0)
    body = y1 * Src1

    def _ref(in0, in1, c0, c1, c2):
        xs = np.ascontiguousarray(in0.astype(np.float32) + np.float32(c0))
        nx = (~xs.view(np.int32)).view(np.float32)
        y0 = nx * np.float32(c1)
        y1 = y0 * (np.float32(c2) - xs * y0)
        return (y1 * in1.astype(np.float32)).astype(np.float32)

    spec = Spec(body=body, reference=_ref)
    shas = {}
    for ver in ("v3", "v4"):
        try:
            s = DveOpSpec(name=name, opcode=None, uops=lower(spec, ver=ver),
                          rd1_en=True)
            shas[ver] = s.sha(ver)
        except Exception:
            pass
    op = dvo.DveOp(name, spec, subdim=False, uops_sha=shas)
    if name not in dvo._SUB_OPCODE_FOR_NAME:
        dvo.OPS.append(op)
        dvo._SUB_OPCODE_FOR_NAME[name] = dvo._CUSTOM_DVE_ROW_BASE + len(dvo.OPS) - 1
        dvo.CUSTOM_DVE_SPECS[name] = spec
        assert dvo._SUB_OPCODE_FOR_NAME[name] < 0x20
    _RECIP_OP = op
    return op


# Chebyshev-minimax seed constants (same as RECIPROCAL_APPROX_FAST)
_RC0 = -0.23549792
_RC1 = 2.0017324


def build():
    recip_op = _get_recip_op()
    nc = bacc.Bacc("TRN2", target_bir_lowering=False, debug=False)
    x_d = nc.dram_tensor("x", [H, C, W], F16, kind="ExternalInput")
    y_d = nc.dram_tensor("y", [H, C, W], F16, kind="ExternalOutput")

    bands_d = nc.inline_tensor(_bands_np().astype(MM_NP), "bands")

    xh = x_d.ap()   # [H, C, W], partition = image row
    yh = y_d.ap()

    with tile.TileContext(nc) as tc, ExitStack() as ctx:
        const = ctx.enter_context(tc.tile_pool(name="const", bufs=1))
        pp = ctx.enter_context(tc.tile_pool(name="pp", bufs=2))
        dp = ctx.enter_context(tc.tile_pool(name="dp", bufs=2))
        gp = ctx.enter_context(tc.tile_pool(name="gp", bufs=2))
        hp = ctx.enter_context(tc.tile_pool(name="hp", bufs=2))
        fin = ctx.enter_context(tc.tile_pool(name="fin", bufs=2))
        psp = ctx.enter_context(tc.tile_pool(name="psp", bufs=1, space="PSUM"))

        # --- constants (issued after tile 0's loads; see loop top) ---
        bands_t = const.tile([P, len(BAND_NAMES), P], MM_DT, tag="bands")
        B = {k: bands_t[:, i, :] for i, k in enumerate(BAND_NAMES)}

        tiles = {}

        def issue_loads(t):
            r0 = t * P
            pma = pp.tile([P, C, WP], F16, tag="pma", bufs=4, name=f"pma_{t}")
            nc.sync.dma_start(out=pma[:, :, IM0 : IM0 + W], in_=xh[r0 : r0 + P])
            pda = pp.tile([P, C, WP], F16, tag="pda", bufs=3, name=f"pda_{t}")
            if t < NT - 1:
                nc.sync.dma_start(out=pda[:, :, IM0 : IM0 + W], in_=xh[r0 + 1 : r0 + P + 1])
            else:
                nc.sync.dma_start(out=pda[: P - 1, :, IM0 : IM0 + W], in_=xh[r0 + 1 : H])
                # reflect: image row 512 -> row 510
                nc.sync.dma_start(out=pda[P - 1 : P, :, IM0 : IM0 + W], in_=xh[H - 2 : H - 1])
            tiles[t] = (pma, pda)

        prev_g = None
        prev_h = None
        prev_evac = None   # (den_ps, a_ps, pmid, r0) of previous tile
        for t in range(NT + 1):
            if t == 0:
                nc.sync.dma_start(out=bands_t, in_=bands_d.ap())
                issue_loads(0)
                # preload the erf_derivative ACT table during the DMA window
                # (reads its own uninitialized tile: value is discarded, only
                # the PSEUDO_LOAD_ACT_FUNC_SET side effect matters)
                act_scr = const.tile([P, 2], F16, tag="act_scr")
                nc.scalar.activation(act_scr, act_scr,
                                     AF.Derivative_Erf, bias=0.0, scale=SCALE_SQ)
                # absorb the bands DMA wait on PE once, then keep the PE busy
                # with dummy matmuls so the HAM clock-gate is warm (2.4 GHz)
                # when the real chains arrive
                ps_scr = psp.tile([P, W], F32, tag="den0", bufs=2, name="ps_scr")
                nc.tensor.matmul(ps_scr[:, :P], B["b_i"], B["b_i"], start=True, stop=True)
                for _ in range(22):
                    nc.tensor.matmul(ps_scr, B["b_i"], bands_t[:, 0:4, :],
                                     start=True, stop=True)
            if t + 1 < NT:
                issue_loads(t + 1)

            if t < NT:
                r0 = t * P
                pma, pda = tiles.pop(t)
                cen = pma[:, :, J1 : J1 + W]

                # --- D_k = P(x+k) - P(x), fp16, all 4-byte aligned -> DVE 2x.
                # S first: it needs neither a B-parity copy nor a pad mirror,
                # so the ACT chain starts as early as possible.  B-parity
                # copies (alignment-free 2x_2P) interleave with the subs.
                d = {}
                for name in ("e", "s", "se", "sw"):
                    d[name] = dp.tile([P, C, WP], F16, tag=f"d_{name}", name=f"d_{name}_{t}")
                if t <= 1:
                    for name in ("e", "s", "se", "sw"):
                        nc.gpsimd.memset(d[name][:, :, 0:J1], 0.0)
                        nc.gpsimd.memset(d[name][:, :, J1 + W : WP], 0.0)
                nc.vector.tensor_sub(d["s"][:, :, J1 : J1 + W], pda[:, :, IM0 : IM0 + W], cen)
                pmb = pp.tile([P, C, WP], F16, tag="pmb", bufs=2, name=f"pmb_{t}")
                pdb = pp.tile([P, C, WP], F16, tag="pdb", bufs=2, name=f"pdb_{t}")
                nc.vector.tensor_copy(pmb[:, :, IMB : IMB + W], pma[:, :, IM0 : IM0 + W])
                nc.gpsimd.tensor_copy(pmb[:, :, IMB + W : IMB + W + 1],
                                      pmb[:, :, IMB + W - 2 : IMB + W - 1])
                nc.vector.tensor_copy(pdb[:, :, IMB : IMB + W], pda[:, :, IM0 : IM0 + W])
                nc.gpsimd.tensor_copy(pdb[:, :, IMB - 1 : IMB],
                                      pdb[:, :, IMB + 1 : IMB + 2])
                nc.gpsimd.tensor_copy(pdb[:, :, IMB + W : IMB + W + 1],
                                      pdb[:, :, IMB + W - 2 : IMB + W - 1])
                nc.vector.tensor_sub(d["e"][:, :, J1 : J1 + W], pmb[:, :, IMB + 1 : IMB + 1 + W], cen)
                nc.vector.tensor_sub(d["se"][:, :, J1 : J1 + W], pdb[:, :, IMB + 1 : IMB + 1 + W], cen)
                nc.vector.tensor_sub(d["sw"][:, :, J1 : J1 + W], pdb[:, :, IMB - 1 : IMB - 1 + W], cen)
                # pad mirrors (tiny; GPSIMD):  D_E(h,-1) = -D_E(h,0);
                # D_SE(h,-1) = D_SW(h,1);  D_SW(h,W) = D_SE(h,W-2)
                nc.gpsimd.tensor_scalar_mul(d["e"][:, :, J0 : J0 + 1],
                                            d["e"][:, :, J1 : J1 + 1], -1.0)
                nc.gpsimd.tensor_copy(d["se"][:, :, J0 : J0 + 1], d["sw"][:, :, J2 : J2 + 1])
                nc.gpsimd.tensor_copy(d["sw"][:, :, J2 + W - 1 : J2 + W],
                                      d["se"][:, :, J2 + W - 3 : J2 + W - 2])

            if t < NT:
                # --- kappa*exp(-D^2/(2s^2)) in ONE ACT pass per offset ---
                g, h = {}, {}
                for name in ("s", "e", "se", "sw"):
                    gk = gp.tile([P, C, WP], MM_DT, tag=f"g_{name}", name=f"g_{name}_{t}")
                    nc.scalar.activation(gk, d[name], AF.Derivative_Erf,
                                         bias=0.0, scale=SCALE_SQ)
                    g[name] = gk

            if t >= 1:
                # --- evac of previous tile: y = c + A * recip1(den + ws0) ---
                # ACT part (a_sb) sits AFTER DerivErf(t) in the scalar FIFO so
                # its wait on PE(t-1) never delays the G chain.
                pden, pa, ppm, pr0 = prev_evac
                a_sb = fin.tile([P, C, W], F32, tag="a_sb", name=f"a_sb_{t-1}")
                for c in range(C):
                    nc.scalar.copy(a_sb[:, c, :], pa[c])

            if t >= 1:
                # channel-0 evac first: releases the single-buffered a0 PSUM
                # bank before PE(t) reaches its A chains
                t8 = fin.tile([P, C, W], F16, tag="t8", name=f"t8_{t-1}")
                nc.vector._custom_dve(
                    recip_op, out=t8[:, 0, :], in0=pden[0], in1=a_sb[:, 0, :],
                    s0=WS0, s1=_RC0, imm2=_RC1)

            if t < NT:
                # --- H~_k = D_k * E_k (DVE TT, fp16 2x; ws folded in bands) ---
                for name in ("s", "e", "se", "sw"):
                    hk = hp.tile([P, C, WP], MM_DT, tag=f"h_{name}", name=f"h_{name}_{t}")
                    nc.vector.tensor_mul(hk, d[name], g[name])
                    h[name] = hk

            if t >= 1:
                # rest of the evac after H(t) in the vector FIFO
                yt = fin.tile([P, C, W], F16, tag="yt", name=f"yt_{t-1}")
                if t - 1 == NT - 1:
                    # last tile: per-channel finish + split stores so the
                    # final (smallest) store starts as early as possible
                    nc.vector.tensor_add(yt[:, 0, :], t8[:, 0, :],
                                         ppm[:, 0, J1 : J1 + W])
                    nc.scalar.dma_start(out=yh[pr0 : pr0 + P, 0], in_=yt[:, 0, :])
                    for c in range(1, C):
                        nc.vector._custom_dve(
                            recip_op, out=t8[:, c, :], in0=pden[c], in1=a_sb[:, c, :],
                            s0=WS0, s1=_RC0, imm2=_RC1)
                        nc.vector.tensor_add(yt[:, c, :], t8[:, c, :],
                                             ppm[:, c, J1 : J1 + W])
                        nc.scalar.dma_start(out=yh[pr0 : pr0 + P, c], in_=yt[:, c, :])
                else:
                    for c in range(1, C):
                        nc.vector._custom_dve(
                            recip_op, out=t8[:, c, :], in0=pden[c], in1=a_sb[:, c, :],
                            s0=WS0, s1=_RC0, imm2=_RC1)
                    nc.vector.tensor_add(yt, t8, ppm[:, :, J1 : J1 + W])
                    nc.scalar.dma_start(out=yh[pr0 : pr0 + P], in_=yt)

            if t < NT:
                # --- PSUM accumulation chains (PE, fp16) ---
                den_ps = [psp.tile([P, W], F32, tag=f"den{c}", name=f"den{c}_{t}",
                                    bufs=2 if c <= 1 else 1) for c in range(C)]
                a_ps = [psp.tile([P, W], F32, tag=f"a{c}", name=f"a{c}_{t}")
                        for c in range(C)]

                def sl(ap, c, j):
                    return ap[:, c, j : j + W]

                gE, gS, gSE, gSW = g["e"], g["s"], g["se"], g["sw"]
                hE, hS, hSE, hSW = h["e"], h["s"], h["se"], h["sw"]
                for c in range(C):
                    dn = den_ps[c]
                    # den chain (ws0 folded into the evac custom op; spatial
                    # weights folded into the bands)
                    nc.tensor.matmul(dn, B["b_isE"], sl(gS, c, J1), start=True, stop=False)
                    nc.tensor.matmul(dn, B["b_iE"], sl(gE, c, J1), start=False, stop=False)
                    nc.tensor.matmul(dn, B["b_iE"], sl(gE, c, J0), start=False, stop=False)
                    nc.tensor.matmul(dn, B["b_iK"], sl(gSE, c, J1), start=False, stop=False)
                    nc.tensor.matmul(dn, B["b_sK"], sl(gSE, c, J0), start=False, stop=False)
                    nc.tensor.matmul(dn, B["b_iK"], sl(gSW, c, J1), start=False, stop=False)
                    nc.tensor.matmul(dn, B["b_sK"], sl(gSW, c, J2), start=False, stop=False)
                    if t == 0:
                        nc.tensor.matmul(dn, B["b_sel0E"], sl(gS, c, J1), start=False, stop=False)
                        nc.tensor.matmul(dn, B["b_sel0K"], sl(gSE, c, J1), start=False, stop=False)
                        nc.tensor.matmul(dn, B["b_sel0K"], sl(gSW, c, J1), start=False, stop=True)
                    else:
                        pgS, pgSE, pgSW = prev_g["s"], prev_g["se"], prev_g["sw"]
                        nc.tensor.matmul(dn, B["b_selE"], sl(pgS, c, J1), start=False, stop=False)
                        nc.tensor.matmul(dn, B["b_selK"], sl(pgSE, c, J0), start=False, stop=False)
                        nc.tensor.matmul(dn, B["b_selK"], sl(pgSW, c, J2), start=False, stop=True)
                for c in range(C):
                    # A chain (spatial weights folded into the bands)
                    an = a_ps[c]
                    nc.tensor.matmul(an, B["b_ainsE"], sl(hS, c, J1), start=True, stop=False)
                    nc.tensor.matmul(an, B["b_aiE"], sl(hE, c, J1), start=False, stop=False)
                    nc.tensor.matmul(an, B["b_aniE"], sl(hE, c, J0), start=False, stop=False)
                    nc.tensor.matmul(an, B["b_aiK"], sl(hSE, c, J1), start=False, stop=False)
                    nc.tensor.matmul(an, B["b_ansK"], sl(hSE, c, J0), start=False, stop=False)
                    nc.tensor.matmul(an, B["b_aiK"], sl(hSW, c, J1), start=False, stop=False)
                    nc.tensor.matmul(an, B["b_ansK"], sl(hSW, c, J2), start=False, stop=False)
                    if t == 0:
                        nc.tensor.matmul(an, B["b_asel0E"], sl(hS, c, J1), start=False, stop=False)
                        nc.tensor.matmul(an, B["b_asel0K"], sl(hSE, c, J1), start=False, stop=False)
                        nc.tensor.matmul(an, B["b_asel0K"], sl(hSW, c, J1), start=False, stop=True)
                    else:
                        phS, phSE, phSW = prev_h["s"], prev_h["se"], prev_h["sw"]
                        nc.tensor.matmul(an, B["b_anselE"], sl(phS, c, J1), start=False, stop=False)
                        nc.tensor.matmul(an, B["b_anselK"], sl(phSE, c, J0), start=False, stop=False)
                        nc.tensor.matmul(an, B["b_anselK"], sl(phSW, c, J2), start=False, stop=True)

                prev_g, prev_h = g, h
                prev_evac = (den_ps, a_ps, pma, r0)

    nc.compile()
    return nc


_NC_CACHE = None


def _get_nc():
    global _NC_CACHE
    if _NC_CACHE is None:
        _NC_CACHE = build()
    return _NC_CACHE


def kernel(batch_img: np.ndarray) -> np.ndarray:
    assert batch_img.shape == (8, C, H, W), batch_img.shape
    # host-side prep: fp16 + [H, C, W] layout per image
    x = np.ascontiguousarray(
        np.asarray(batch_img, dtype=np.float16).transpose(0, 2, 1, 3))
    nc = _get_nc()
    in_maps = [{"x": x[b]} for b in range(8)]
    r = run_bass_kernel_spmd(nc, in_maps, core_ids=list(range(8)))
    out = np.stack([r.results[b]["y"] for b in range(8)], axis=0)  # [8,H,C,W]
    return np.ascontiguousarray(out.transpose(0, 2, 1, 3)).astype(np.float32)


if __name__ == "__main__":
    rng = np.random.default_rng(0)
    img = rng.random((8, C, H, W), np.float32)
    y = kernel(img)
    print("ran ok", y.shape, y.dtype)
